# revision 30
# baseline (speedup 1.0000x reference)
"""Trainium2 Bass kernel: multi-head elementwise-attention GNN message passing.

Full inputs -> full output. Edges partitioned by destination-node block across
8 NeuronCores. Per core:
  Phase A: kv = [x@Wk | x@Wv] (+bias) in bf16 for ALL nodes, written to two
           DRAM tables (lo/hi node halves so gather indices fit int16).
  Phase B: q = x@Wq (+bias) in bf16 for the core's own nodes -> DRAM table.
  Phase C: per chunk of blocks, three dma_gather ops fetch per-edge k|v rows
           (by src) and q rows (by dst) into SBUF bf16; per tile of 128 edges:
           t1 = k*q (DVE 2x), m = exp(t1/4) (ACT), mv = m*v (DVE 2x),
           one-hot dst-offset columns via tensor_scalar is_equal (DVE 4x),
           z/num segment sums as bf16 matmuls accumulated in PSUM.
Out = (num/z) @ Wo done per block; bo added on host.
"""
import sys
sys.path.insert(0, '/opt/trn_rl_repo')
import math
import numpy as np
import ml_dtypes

import concourse.bass as bass
import concourse.bacc as bacc
import concourse.mybir as mybir
import concourse.tile as tile
from concourse import bass2jax

P = 128
D = 128
N_CORES = 8
CHUNK = 3  # dst blocks per gather chunk
GRP = 6    # tiles per DVE/ACT batch

f32 = mybir.dt.float32
bf16 = mybir.dt.bfloat16
i16 = mybir.dt.int16

_cache = {}


def _build(layout, n_all_blk, half_rows, with_bias, ablate='', repeat=1):
    """layout: tuple of (t_lo_j, t_hi_j) per owned block (uniform across cores).
    n_all_blk: total node blocks (kv table rows = n_all_blk*P, split lo/hi).
    half_rows: rows per kv half-table."""
    key = (layout, n_all_blk, half_rows, with_bias, ablate, repeat)
    if key in _cache:
        return _cache[key]
    nblk_core = len(layout)
    n_core = nblk_core * P
    total_tiles = sum(tl + th for tl, th in layout)
    total_lo = sum(tl for tl, th in layout)
    total_hi = sum(th for tl, th in layout)

    # chunk partitioning of the 49 blocks
    chunks = []
    j = 0
    while j < nblk_core:
        chunks.append(list(range(j, min(j + CHUNK, nblk_core))))
        j += CHUNK

    nc = bacc.Bacc("TRN2", target_bir_lowering=False, debug=False,
                   num_devices=N_CORES)
    # ---- I/O ----
    xT = nc.dram_tensor("xT", [P, n_all_blk * P], bf16, kind="ExternalInput")
    xTq = nc.dram_tensor("xTq", [P, n_core], bf16, kind="ExternalInput")
    wkv = nc.dram_tensor("wkv", [D, 2 * D], bf16, kind="ExternalInput")
    wq = nc.dram_tensor("wq", [D, D], bf16, kind="ExternalInput")
    wo = nc.dram_tensor("wo", [D, D], bf16, kind="ExternalInput")
    bkv = nc.dram_tensor("bkv", [1, 2 * D], bf16, kind="ExternalInput")
    bqr = nc.dram_tensor("bqr", [1, D], bf16, kind="ExternalInput")
    ones1 = nc.dram_tensor("ones1", [1, P], bf16, kind="ExternalInput")
    iotaF = nc.dram_tensor("iotaF", [P, GRP * P], bf16, kind="ExternalInput")
    loidx = nc.dram_tensor("loidx", [P, total_lo * 8], i16, kind="ExternalInput")
    hiidx = nc.dram_tensor("hiidx", [P, max(total_hi, 1) * 8], i16,
                           kind="ExternalInput")
    qidx = nc.dram_tensor("qidx", [P, total_tiles * 8], i16, kind="ExternalInput")
    occ = nc.dram_tensor("occ", [P, total_tiles], f32, kind="ExternalInput")
    outT = nc.dram_tensor("outT", [P, n_core], f32, kind="ExternalOutput")

    inv_sqrt_dk = 1.0 / math.sqrt(D // 8)  # d_k = 16

    with tile.TileContext(nc) as tc:
        with tc.tile_pool(name="const", bufs=1) as cp, \
             tc.tile_pool(name="dram", bufs=1, space="DRAM") as dp, \
             tc.tile_pool(name="xld", bufs=4) as xp, \
             tc.tile_pool(name="kvw", bufs=4) as kp, \
             tc.tile_pool(name="meta", bufs=2) as mp, \
             tc.tile_pool(name="gath", bufs=2) as gp, \
             tc.tile_pool(name="work", bufs=4) as wp, \
             tc.tile_pool(name="epi", bufs=3) as ep, \
             tc.tile_pool(name="mm", bufs=2, space="PSUM") as pp, \
             tc.tile_pool(name="acc", bufs=1, space="PSUM") as pz:

            # ---- constants ----
            wkv_s = cp.tile([D, 2 * D], bf16)
            nc.sync.dma_start(out=wkv_s[:], in_=wkv.ap())
            wq_s = cp.tile([D, D], bf16)
            nc.sync.dma_start(out=wq_s[:], in_=wq.ap())
            wo_s = cp.tile([D, D], bf16)
            nc.sync.dma_start(out=wo_s[:], in_=wo.ap())
            on_s = cp.tile([1, P], bf16)
            nc.sync.dma_start(out=on_s[:], in_=ones1.ap())
            bkv_s = cp.tile([1, 2 * D], bf16)
            nc.sync.dma_start(out=bkv_s[:], in_=bkv.ap())
            bq_s = cp.tile([1, D], bf16)
            nc.sync.dma_start(out=bq_s[:], in_=bqr.ap())
            iF_s = cp.tile([P, GRP * P], bf16)
            nc.sync.dma_start(out=iF_s[:], in_=iotaF.ap())
            zb_s = cp.tile([P, 1], f32)
            nc.vector.memset(zb_s[:], 0.0)

            kv_lo = dp.tile([half_rows, 2 * D], bf16)
            kv_hi = dp.tile([half_rows, 2 * D], bf16)
            q_dram = dp.tile([n_core, D], bf16)

            for _rep in range(repeat):
                # ---- Phase B first: q for own nodes -> DRAM (unblocks
                # q-gathers early) ----
                XCH = 8  # blocks per x load
                for j0 in range(0, nblk_core, XCH):
                    jn = min(XCH, nblk_core - j0)
                    xt = xp.tile([P, XCH * P], bf16, tag="xt")
                    nc.sync.dma_start(
                        out=xt[:, 0:jn * P],
                        in_=xTq.ap()[:, j0 * P:(j0 + jn) * P])
                    for j in range(j0, j0 + jn, 2):
                        pn = min(2, j0 + jn - j)
                        pq = pp.tile([P, 4 * D], f32, tag="mm")
                        for k in range(pn):
                            nc.tensor.matmul(
                                out=pq[:, k * 2 * D:k * 2 * D + D],
                                lhsT=xt[:, (j - j0 + k) * P:(j - j0 + k + 1) * P],
                                rhs=wq_s[:], start=True, stop=not with_bias)
                            if with_bias:
                                nc.tensor.matmul(
                                    out=pq[:, k * 2 * D:k * 2 * D + D],
                                    lhsT=on_s[:], rhs=bq_s[:],
                                    start=False, stop=True)
                        q_t = kp.tile([P, 2 * D], bf16, tag="qw")
                        for k in range(pn):
                            if (j // 2) % 2 == 0:
                                nc.vector.tensor_copy(
                                    out=q_t[:, k * D:(k + 1) * D],
                                    in_=pq[:, k * 2 * D:k * 2 * D + D])
                            else:
                                nc.scalar.copy(
                                    out=q_t[:, k * D:(k + 1) * D],
                                    in_=pq[:, k * 2 * D:k * 2 * D + D])
                        nc.sync.dma_start(
                            out=q_dram[j * P:(j + pn) * P, :].rearrange(
                                "(i p) c -> p i c", p=P),
                            in_=q_t[:, 0:pn * D].rearrange(
                                "p (i c) -> p i c", c=D))

                # ---- Phase A: kv for ALL nodes -> DRAM lo/hi ----
                half_blk = half_rows // P
                for b0 in range(0, n_all_blk, XCH):
                    bn = min(XCH, n_all_blk - b0)
                    xt = xp.tile([P, XCH * P], bf16, tag="xt")
                    nc.sync.dma_start(
                        out=xt[:, 0:bn * P],
                        in_=xT.ap()[:, b0 * P:(b0 + bn) * P])
                    for b in range(b0, b0 + bn, 2):
                        pn = min(2, b0 + bn - b)
                        pkv = pp.tile([P, 4 * D], f32, tag="mm")
                        for k in range(pn):
                            nc.tensor.matmul(
                                out=pkv[:, k * 2 * D:(k + 1) * 2 * D],
                                lhsT=xt[:, (b - b0 + k) * P:(b - b0 + k + 1) * P],
                                rhs=wkv_s[:], start=True, stop=not with_bias)
                            if with_bias:
                                nc.tensor.matmul(
                                    out=pkv[:, k * 2 * D:(k + 1) * 2 * D],
                                    lhsT=on_s[:], rhs=bkv_s[:],
                                    start=False, stop=True)
                        kv_t = kp.tile([P, 4 * D], bf16, tag="kvw")
                        if (b // 2) % 2 == 0:
                            nc.vector.tensor_copy(out=kv_t[:, 0:pn * 2 * D],
                                                  in_=pkv[:, 0:pn * 2 * D])
                        else:
                            nc.scalar.copy(out=kv_t[:, 0:pn * 2 * D],
                                           in_=pkv[:, 0:pn * 2 * D])
                        # both blocks of a pair land in the same half-table
                        dst = kv_lo if b < half_blk else kv_hi
                        bb = b if b < half_blk else b - half_blk
                        nc.sync.dma_start(
                            out=dst[bb * P:(bb + pn) * P, :].rearrange(
                                "(i p) c -> p i c", p=P),
                            in_=kv_t[:, 0:pn * 2 * D].rearrange(
                                "p (i c) -> p i c", c=2 * D))

                # ---- Phase C ----
                off_lo = 0   # in tiles, global across chunks
                off_hi = 0
                off_t = 0    # tile offset into qidx/occ
                for ch in (chunks if 'noc' not in ablate else []):
                    c_lo = sum(layout[j][0] for j in ch)
                    c_hi = sum(layout[j][1] for j in ch)
                    tk = c_lo + c_hi

                    # metadata loads
                    oc_t = mp.tile([P, tk], f32, tag="oc")
                    nc.sync.dma_start(out=oc_t[:],
                                      in_=occ.ap()[:, off_t:off_t + tk])
                    qx_t = mp.tile([P, tk * 8], i16, tag="qx")
                    nc.sync.dma_start(out=qx_t[:],
                                      in_=qidx.ap()[:, off_t * 8:(off_t + tk) * 8])
                    if c_lo:
                        lx_t = mp.tile([P, c_lo * 8], i16, tag="lx")
                        nc.sync.dma_start(
                            out=lx_t[:],
                            in_=loidx.ap()[:, off_lo * 8:(off_lo + c_lo) * 8])
                    if c_hi:
                        hx_t = mp.tile([P, c_hi * 8], i16, tag="hx")
                        nc.sync.dma_start(
                            out=hx_t[:],
                            in_=hiidx.ap()[:, off_hi * 8:(off_hi + c_hi) * 8])

                    # gathers
                    kvg = gp.tile([P, tk * 2 * D], bf16, tag="kvg")
                    kvg3 = kvg[:].rearrange("p (t e) -> p t e", e=2 * D)
                    qg = gp.tile([P, tk * D], bf16, tag="qg")
                    qg3 = qg[:].rearrange("p (t e) -> p t e", e=D)
                    if 'nogather' in ablate:
                        for t in range(tk):
                            nc.sync.dma_start(out=kvg3[:, t, :],
                                              in_=kv_lo[0:P, :])
                            nc.sync.dma_start(out=qg3[:, t, :],
                                              in_=q_dram[0:P, :])
                    elif 'gone' in ablate:
                        # one full-out kv gather (wrong values, bounds-safe)
                        nc.gpsimd.dma_gather(
                            kvg3[:, :, :], kv_lo[:], qx_t[:],
                            tk * P, tk * P, 2 * D, single_packet=False)
                        nc.gpsimd.dma_gather(
                            qg3[:, :, :],
                            q_dram[:], qx_t[:], tk * P, tk * P, D, single_packet=False)
                    elif 'gq' in ablate:
                        for t in range(tk):
                            nc.sync.dma_start(out=kvg3[:, t, :],
                                              in_=kv_lo[0:P, :])
                        nc.gpsimd.dma_gather(
                            qg3[:, :, :],
                            q_dram[:], qx_t[:], tk * P, tk * P, D, single_packet=False)
                    elif 'gkv' in ablate:
                        nc.gpsimd.dma_gather(
                            kvg3[:, :, :], kv_lo[:], qx_t[:],
                            tk * P, tk * P, 2 * D, single_packet=False)
                        for t in range(tk):
                            nc.sync.dma_start(out=qg3[:, t, :],
                                              in_=q_dram[0:P, :])
                    else:
                        if c_lo:
                            nc.gpsimd.dma_gather(
                                kvg3[:, 0:c_lo, :], kv_lo[:], lx_t[:],
                                c_lo * P, c_lo * P, 2 * D, single_packet=False)
                        if c_hi:
                            nc.gpsimd.dma_gather(
                                kvg3[:, c_lo:tk, :], kv_hi[:], hx_t[:],
                                c_hi * P, c_hi * P, 2 * D, single_packet=False)
                        nc.gpsimd.dma_gather(
                            qg3[:, :, :],
                            q_dram[:], qx_t[:], tk * P, tk * P, D, single_packet=False)

                    if 'gonly' in ablate:
                        off_lo += c_lo
                        off_hi += c_hi
                        off_t += tk
                        continue

                    # per-block psum accumulators
                    zt = {j: pz.tile([P, P], f32, tag=f"z{j % CHUNK}",
                                     name=f"zt{j % CHUNK}") for j in ch}
                    ntl = {j: pz.tile([P, P], f32, tag=f"n{j % CHUNK}",
                                      name=f"ntl{j % CHUNK}") for j in ch}

                    # tile -> (block, first?, last?) in chunk order
                    tmap = []
                    for reg in (0, 1):  # lo region then hi region
                        for j in ch:
                            tl, th = layout[j]
                            cnt = tl if reg == 0 else th
                            for _ in range(cnt):
                                tmap.append(j)
                    first_seen, last_idx = {}, {}
                    for t, j in enumerate(tmap):
                        first_seen.setdefault(j, t)
                        last_idx[j] = t

                    # process tiles in groups
                    t = 0
                    while t < tk:
                        g = min(GRP, tk - t)
                        t1 = wp.tile([P, GRP * D], bf16, tag="t1")
                        nc.vector.tensor_tensor(
                            out=t1[:, 0:g * D].rearrange("p (t c) -> p t c", t=g),
                            in0=kvg3[:, t:t + g, 0:D],
                            in1=qg3[:, t:t + g, :],
                            op=mybir.AluOpType.mult)
                        m_t = wp.tile([P, GRP * D], bf16, tag="m")
                        if 'noexp' in ablate:
                            nc.vector.tensor_copy(out=m_t[:, 0:g * D],
                                                  in_=t1[:, 0:g * D])
                        else:
                            nc.scalar.activation(
                                m_t[:, 0:g * D], t1[:, 0:g * D],
                                mybir.ActivationFunctionType.Exp,
                                bias=zb_s[:], scale=inv_sqrt_dk)
                        mv_t = wp.tile([P, GRP * D], bf16, tag="mv")
                        nc.vector.tensor_tensor(
                            out=mv_t[:, 0:g * D].rearrange("p (t c) -> p t c", t=g),
                            in0=m_t[:, 0:g * D].rearrange("p (t c) -> p t c", t=g),
                            in1=kvg3[:, t:t + g, D:2 * D],
                            op=mybir.AluOpType.mult)
                        s_t = wp.tile([P, GRP * P], bf16, tag="ssc")
                        if 'noss' in ablate:
                            pass
                        elif 'ssscalar' not in ablate:
                            nc.vector.tensor_tensor(
                                out=s_t[:, 0:g * P].rearrange(
                                    "p (t c) -> p t c", t=g),
                                in0=iF_s[:, 0:g * P].rearrange(
                                    "p (t c) -> p t c", t=g),
                                in1=oc_t[:, t:t + g].to_broadcast([P, g, P]),
                                op=mybir.AluOpType.is_equal)
                        else:
                            for i in range(g):
                                nc.vector.tensor_scalar(
                                    out=s_t[:, i * P:(i + 1) * P],
                                    in0=iF_s[:, 0:P],
                                    scalar1=oc_t[:, t + i:t + i + 1], scalar2=None,
                                    op0=mybir.AluOpType.is_equal)
                        if 'nomm' not in ablate:
                            src_s = iF_s if 'noss' in ablate else s_t
                            for i in range(g):
                                ti = t + i
                                j = tmap[ti]
                                st = first_seen[j] == ti
                                sp = last_idx[j] == ti
                                si = 0 if 'noss' in ablate else i * P
                                nc.tensor.matmul(out=zt[j][:],
                                                 lhsT=m_t[:, i * D:(i + 1) * D],
                                                 rhs=src_s[:, si:si + P],
                                                 start=st, stop=sp)
                                nc.tensor.matmul(out=ntl[j][:],
                                                 lhsT=mv_t[:, i * D:(i + 1) * D],
                                                 rhs=src_s[:, si:si + P],
                                                 start=st, stop=sp)
                        t += g

                    # epilogue per block
                    for j in (ch if ('noepi' not in ablate
                                     and 'nomm' not in ablate) else []):
                        rz = ep.tile([P, P], f32, tag="rz")
                        nc.vector.reciprocal(out=rz[:], in_=zt[j][:])
                        ox = ep.tile([P, P], bf16, tag="ox")
                        nc.vector.tensor_tensor(out=ox[:], in0=ntl[j][:],
                                                in1=rz[:],
                                                op=mybir.AluOpType.mult)
                        po = pp.tile([P, 2 * D], f32, tag="mm")
                        nc.tensor.matmul(out=po[:, 0:P], lhsT=wo_s[:], rhs=ox[:],
                                         start=True, stop=True)
                        o_sb = ep.tile([P, P], f32, tag="osb")
                        nc.vector.tensor_copy(out=o_sb[:], in_=po[:, 0:P])
                        nc.sync.dma_start(out=outT.ap()[:, j * P:(j + 1) * P],
                                          in_=o_sb[:])

                    off_lo += c_lo
                    off_hi += c_hi
                    off_t += tk

    nc.compile()
    _cache[key] = nc
    return nc


def _wrap_idx(idx_flat):
    """Pack a flat idx list into the [128, ceil(n/16)] int16 layout:
    idx i -> partition i%16, col i//16; replicated to the 8 groups of 16."""
    n = idx_flat.shape[0]
    cols = (n + 15) // 16
    arr = np.zeros((16, cols), np.int16)
    arr[
        np.arange(n) % 16, np.arange(n) // 16
    ] = idx_flat.astype(np.int16)
    return np.tile(arr, (8, 1))


def kernel(x, src, dst, Wq, bq, Wk, bk, Wv, bv, Wo, bo):
    x = np.asarray(x, dtype=np.float32)
    n, d = x.shape
    assert d == D
    src = np.asarray(src, dtype=np.int64)
    dst = np.asarray(dst, dtype=np.int64)

    n_all_blk = math.ceil(n / P)
    n_all_blk = math.ceil(n_all_blk / N_CORES) * N_CORES
    # ensure even number of blocks for lo/hi halves
    if n_all_blk % 2:
        n_all_blk += N_CORES
    n_pad = n_all_blk * P
    nblk_core = n_all_blk // N_CORES
    n_core = nblk_core * P
    half_blk = n_all_blk // 2
    half_rows = half_blk * P
    assert half_rows - 1 <= np.iinfo(np.int16).max
    assert n_core - 1 <= np.iinfo(np.int16).max

    # ---- host prep: sort edges by dst block, split by src half ----
    order = np.argsort(dst, kind="stable")
    sdst = dst[order]
    ssrc = src[order]
    blk = sdst // P
    counts = np.bincount(blk, minlength=n_all_blk)
    starts = np.zeros(n_all_blk + 1, dtype=np.int64)
    np.cumsum(counts, out=starts[1:])

    is_lo = ssrc < half_rows
    # per block lo/hi counts
    nlo = np.zeros(n_all_blk, np.int64)
    for b in range(n_all_blk):
        s0, s1 = starts[b], starts[b + 1]
        nlo[b] = int(is_lo[s0:s1].sum())
    nhi = counts - nlo
    tlo_b = (nlo + P - 1) // P
    thi_b = (nhi + P - 1) // P
    # uniform layout across cores: per block-slot j take max over cores
    tlo_j = [int(max(tlo_b[c * nblk_core + j] for c in range(N_CORES)))
             for j in range(nblk_core)]
    thi_j = [int(max(thi_b[c * nblk_core + j] for c in range(N_CORES)))
             for j in range(nblk_core)]
    layout = tuple((tlo_j[j], thi_j[j]) for j in range(nblk_core))
    total_tiles = sum(tl + th for tl, th in layout)
    total_lo = sum(tl for tl, th in layout)
    total_hi = sum(th for tl, th in layout)

    # chunk structure (must mirror _build)
    chunks = []
    j = 0
    while j < nblk_core:
        chunks.append(list(range(j, min(j + CHUNK, nblk_core))))
        j += CHUNK

    # ---- per-core data ----
    lo_np = np.zeros((N_CORES, P, total_lo * 8), np.int16)
    hi_np = np.zeros((N_CORES, P, max(total_hi, 1) * 8), np.int16)
    qx_np = np.zeros((N_CORES, P, total_tiles * 8), np.int16)
    oc_np = np.full((N_CORES, P, total_tiles), 255.0, np.float32)

    for c in range(N_CORES):
        lo_list, hi_list, q_list, oc_list = [], [], [], []
        # chunk-ordered
        for ch in chunks:
            blo, bhi, bq_, boc = [], [], [], []
            for reg in (0, 1):
                for j in ch:
                    b = c * nblk_core + j
                    s0, s1 = starts[b], starts[b + 1]
                    m = is_lo[s0:s1] if reg == 0 else ~is_lo[s0:s1]
                    es = ssrc[s0:s1][m]
                    ed = sdst[s0:s1][m]
                    ntile = layout[j][reg]
                    npad = ntile * P
                    idx = np.zeros(npad, np.int64)
                    idx[:es.shape[0]] = es if reg == 0 else es - half_rows
                    qi = np.zeros(npad, np.int64)
                    qi[:ed.shape[0]] = ed - c * n_core
                    ocv = np.full(npad, 255.0, np.float32)
                    ocv[:ed.shape[0]] = (ed - b * P).astype(np.float32)
                    (blo if reg == 0 else bhi).append(idx)
                    bq_.append(qi)
                    boc.append(ocv)
            lo_list.append(np.concatenate(blo) if blo else np.zeros(0, np.int64))
            hi_list.append(np.concatenate(bhi) if bhi else np.zeros(0, np.int64))
            q_list.append(np.concatenate(bq_))
            oc_list.append(np.concatenate(boc))
        lo_flat = np.concatenate(lo_list)
        hi_flat = np.concatenate(hi_list)
        q_flat = np.concatenate(q_list)
        oc_flat = np.concatenate(oc_list)
        # pack
        lo_np[c] = _wrap_idx_fill(lo_flat, total_lo * 8)
        hi_np[c] = _wrap_idx_fill(hi_flat, max(total_hi, 1) * 8)
        qx_np[c] = _wrap_idx_fill(q_flat, total_tiles * 8)
        # oc: [P, total_tiles]: slot (tile t, partition p) = edge t*128+p
        oc_np[c] = oc_flat.reshape(total_tiles, P).T

    xb = np.zeros((n_pad, D), np.float32)
    xb[:n] = x
    xT_np = np.ascontiguousarray(xb.T).astype(ml_dtypes.bfloat16)

    wkv_np = np.concatenate([np.asarray(Wk, np.float32),
                             np.asarray(Wv, np.float32)], axis=1) \
        .astype(ml_dtypes.bfloat16)
    wq_np = np.asarray(Wq, np.float32).astype(ml_dtypes.bfloat16)
    wo_np = np.asarray(Wo, np.float32).astype(ml_dtypes.bfloat16)
    bkv_np = np.concatenate([np.asarray(bk, np.float32),
                             np.asarray(bv, np.float32)])[None, :] \
        .astype(ml_dtypes.bfloat16)
    bq_np = np.asarray(bq, np.float32)[None, :].astype(ml_dtypes.bfloat16)
    ones1_np = np.ones((1, P), ml_dtypes.bfloat16)
    iota_np = np.tile(np.arange(P, dtype=np.float32)[None, :],
                      (P, GRP)).astype(ml_dtypes.bfloat16)
    with_bias = bool(np.any(np.asarray(bq)) or np.any(np.asarray(bk))
                     or np.any(np.asarray(bv)))

    import os
    nc = _build(layout, n_all_blk, half_rows, with_bias,
                ablate=os.environ.get("K_ABLATE", ""))

    in_maps = []
    for c in range(N_CORES):
        in_maps.append({
            "xT": xT_np,
            "xTq": np.ascontiguousarray(xT_np[:, c * n_core:(c + 1) * n_core]),
            "wkv": wkv_np, "wq": wq_np, "wo": wo_np,
            "bkv": bkv_np, "bqr": bq_np, "ones1": ones1_np, "iotaF": iota_np,
            "loidx": lo_np[c], "hiidx": hi_np[c], "qidx": qx_np[c],
            "occ": oc_np[c],
        })
    results = bass2jax.run_bass_via_pjrt(nc, in_maps, n_cores=N_CORES)

    out = np.empty((n_pad, D), np.float32)
    for c in range(N_CORES):
        out[c * n_core:(c + 1) * n_core] = results[c]["outT"].T
    out = out[:n] + np.asarray(bo, np.float32)[None, :]
    return out.astype(np.float32)


def _wrap_idx_fill(idx_flat, ncols):
    """_wrap_idx padded to exactly ncols columns."""
    arr = np.zeros((P, ncols), np.int16)
    if idx_flat.size == 0:
        return arr
    w = _wrap_idx(idx_flat)
    arr[:, :w.shape[1]] = w
    return arr


# revision 34
# speedup vs baseline: 1.4226x; 1.4226x over previous
"""Trainium2 Bass kernel: multi-head elementwise-attention GNN message passing.

Full inputs -> full output. Edges partitioned by destination-node block across
8 NeuronCores. Per core:
  Phase A: kv = [x@Wk | x@Wv] (+bias) in bf16 for ALL nodes, written to two
           DRAM tables (lo/hi node halves so gather indices fit int16).
  Phase B: q = x@Wq (+bias) in bf16 for the core's own nodes -> DRAM table.
  Phase C: per chunk of blocks, three dma_gather ops fetch per-edge k|v rows
           (by src) and q rows (by dst) into SBUF bf16; per tile of 128 edges:
           t1 = k*q (DVE 2x), m = exp(t1/4) (ACT), mv = m*v (DVE 2x),
           one-hot dst-offset columns via tensor_scalar is_equal (DVE 4x),
           z/num segment sums as bf16 matmuls accumulated in PSUM.
Out = (num/z) @ Wo done per block; bo added on host.
"""
import sys
sys.path.insert(0, '/opt/trn_rl_repo')
import math
import numpy as np
import ml_dtypes

import concourse.bass as bass
import concourse.bacc as bacc
import concourse.mybir as mybir
import concourse.tile as tile
from concourse import bass2jax

P = 128
D = 128
N_CORES = 8
CHUNK = 3  # dst blocks per gather chunk
GRP = 6    # tiles per DVE/ACT batch

f32 = mybir.dt.float32
bf16 = mybir.dt.bfloat16
i16 = mybir.dt.int16

_cache = {}


def _build(layout, n_all_blk, half_rows, with_bias, ablate='', repeat=1):
    """layout: tuple of (t_lo_j, t_hi_j) per owned block (uniform across cores).
    n_all_blk: total node blocks (kv table rows = n_all_blk*P, split lo/hi).
    half_rows: rows per kv half-table."""
    key = (layout, n_all_blk, half_rows, with_bias, ablate, repeat)
    if key in _cache:
        return _cache[key]
    nblk_core = len(layout)
    n_core = nblk_core * P
    total_tiles = sum(tl + th for tl, th in layout)
    total_lo = sum(tl for tl, th in layout)
    total_hi = sum(th for tl, th in layout)

    # chunk partitioning of the 49 blocks
    chunks = []
    j = 0
    while j < nblk_core:
        chunks.append(list(range(j, min(j + CHUNK, nblk_core))))
        j += CHUNK

    nc = bacc.Bacc("TRN2", target_bir_lowering=False, debug=False,
                   num_devices=N_CORES, num_swdge_queues=4)
    # ---- I/O ----
    xT = nc.dram_tensor("xT", [P, n_all_blk * P], bf16, kind="ExternalInput")
    xTq = nc.dram_tensor("xTq", [P, n_core], bf16, kind="ExternalInput")
    wkv = nc.dram_tensor("wkv", [D, 2 * D], bf16, kind="ExternalInput")
    wq = nc.dram_tensor("wq", [D, D], bf16, kind="ExternalInput")
    wo = nc.dram_tensor("wo", [D, D], bf16, kind="ExternalInput")
    bkv = nc.dram_tensor("bkv", [1, 2 * D], bf16, kind="ExternalInput")
    bqr = nc.dram_tensor("bqr", [1, D], bf16, kind="ExternalInput")
    ones1 = nc.dram_tensor("ones1", [1, P], bf16, kind="ExternalInput")
    iotaF = nc.dram_tensor("iotaF", [P, GRP * P], bf16, kind="ExternalInput")
    loidx = nc.dram_tensor("loidx", [P, total_lo * 8], i16, kind="ExternalInput")
    hiidx = nc.dram_tensor("hiidx", [P, max(total_hi, 1) * 8], i16,
                           kind="ExternalInput")
    qidx = nc.dram_tensor("qidx", [P, total_tiles * 8], i16, kind="ExternalInput")
    occ = nc.dram_tensor("occ", [P, total_tiles], f32, kind="ExternalInput")
    outT = nc.dram_tensor("outT", [P, n_core], f32, kind="ExternalOutput")

    inv_sqrt_dk = 1.0 / math.sqrt(D // 8)  # d_k = 16

    with tile.TileContext(nc) as tc:
        with tc.tile_pool(name="const", bufs=1) as cp, \
             tc.tile_pool(name="dram", bufs=1, space="DRAM") as dp, \
             tc.tile_pool(name="xld", bufs=4) as xp, \
             tc.tile_pool(name="kvw", bufs=4) as kp, \
             tc.tile_pool(name="meta", bufs=2) as mp, \
             tc.tile_pool(name="gath", bufs=2) as gp, \
             tc.tile_pool(name="work", bufs=4) as wp, \
             tc.tile_pool(name="epi", bufs=3) as ep, \
             tc.tile_pool(name="mm", bufs=2, space="PSUM") as pp, \
             tc.tile_pool(name="acc", bufs=1, space="PSUM") as pz:

            # ---- constants ----
            wkv_s = cp.tile([D, 2 * D], bf16)
            nc.sync.dma_start(out=wkv_s[:], in_=wkv.ap())
            wq_s = cp.tile([D, D], bf16)
            nc.sync.dma_start(out=wq_s[:], in_=wq.ap())
            wo_s = cp.tile([D, D], bf16)
            nc.sync.dma_start(out=wo_s[:], in_=wo.ap())
            on_s = cp.tile([1, P], bf16)
            nc.sync.dma_start(out=on_s[:], in_=ones1.ap())
            bkv_s = cp.tile([1, 2 * D], bf16)
            nc.sync.dma_start(out=bkv_s[:], in_=bkv.ap())
            bq_s = cp.tile([1, D], bf16)
            nc.sync.dma_start(out=bq_s[:], in_=bqr.ap())
            iF_s = cp.tile([P, GRP * P], bf16)
            nc.sync.dma_start(out=iF_s[:], in_=iotaF.ap())
            zb_s = cp.tile([P, 1], f32)
            nc.vector.memset(zb_s[:], 0.0)

            kv_lo = dp.tile([half_rows, 2 * D], bf16)
            kv_hi = dp.tile([half_rows, 2 * D], bf16)
            q_dram = dp.tile([n_core, D], bf16)

            for _rep in range(repeat):
                # ---- Phase B first: q for own nodes -> DRAM (unblocks
                # q-gathers early) ----
                XCH = 8  # blocks per x load
                for j0 in range(0, nblk_core, XCH):
                    jn = min(XCH, nblk_core - j0)
                    xt = xp.tile([P, XCH * P], bf16, tag="xt")
                    nc.sync.dma_start(
                        out=xt[:, 0:jn * P],
                        in_=xTq.ap()[:, j0 * P:(j0 + jn) * P])
                    for j in range(j0, j0 + jn, 2):
                        pn = min(2, j0 + jn - j)
                        pq = pp.tile([P, 4 * D], f32, tag="mm")
                        for k in range(pn):
                            nc.tensor.matmul(
                                out=pq[:, k * 2 * D:k * 2 * D + D],
                                lhsT=xt[:, (j - j0 + k) * P:(j - j0 + k + 1) * P],
                                rhs=wq_s[:], start=True, stop=not with_bias)
                            if with_bias:
                                nc.tensor.matmul(
                                    out=pq[:, k * 2 * D:k * 2 * D + D],
                                    lhsT=on_s[:], rhs=bq_s[:],
                                    start=False, stop=True)
                        q_t = kp.tile([P, 2 * D], bf16, tag="qw")
                        for k in range(pn):
                            if (j // 2) % 2 == 0:
                                nc.vector.tensor_copy(
                                    out=q_t[:, k * D:(k + 1) * D],
                                    in_=pq[:, k * 2 * D:k * 2 * D + D])
                            else:
                                nc.scalar.copy(
                                    out=q_t[:, k * D:(k + 1) * D],
                                    in_=pq[:, k * 2 * D:k * 2 * D + D])
                        nc.sync.dma_start(
                            out=q_dram[j * P:(j + pn) * P, :].rearrange(
                                "(i p) c -> p i c", p=P),
                            in_=q_t[:, 0:pn * D].rearrange(
                                "p (i c) -> p i c", c=D))

                # ---- Phase A: kv for ALL nodes -> DRAM lo/hi ----
                half_blk = half_rows // P
                for b0 in range(0, n_all_blk, XCH):
                    bn = min(XCH, n_all_blk - b0)
                    xt = xp.tile([P, XCH * P], bf16, tag="xt")
                    nc.sync.dma_start(
                        out=xt[:, 0:bn * P],
                        in_=xT.ap()[:, b0 * P:(b0 + bn) * P])
                    for b in range(b0, b0 + bn, 2):
                        pn = min(2, b0 + bn - b)
                        pkv = pp.tile([P, 4 * D], f32, tag="mm")
                        for k in range(pn):
                            nc.tensor.matmul(
                                out=pkv[:, k * 2 * D:(k + 1) * 2 * D],
                                lhsT=xt[:, (b - b0 + k) * P:(b - b0 + k + 1) * P],
                                rhs=wkv_s[:], start=True, stop=not with_bias)
                            if with_bias:
                                nc.tensor.matmul(
                                    out=pkv[:, k * 2 * D:(k + 1) * 2 * D],
                                    lhsT=on_s[:], rhs=bkv_s[:],
                                    start=False, stop=True)
                        kv_t = kp.tile([P, 4 * D], bf16, tag="kvw")
                        if (b // 2) % 2 == 0:
                            nc.vector.tensor_copy(out=kv_t[:, 0:pn * 2 * D],
                                                  in_=pkv[:, 0:pn * 2 * D])
                        else:
                            nc.scalar.copy(out=kv_t[:, 0:pn * 2 * D],
                                           in_=pkv[:, 0:pn * 2 * D])
                        # both blocks of a pair land in the same half-table
                        dst = kv_lo if b < half_blk else kv_hi
                        bb = b if b < half_blk else b - half_blk
                        nc.sync.dma_start(
                            out=dst[bb * P:(bb + pn) * P, :].rearrange(
                                "(i p) c -> p i c", p=P),
                            in_=kv_t[:, 0:pn * 2 * D].rearrange(
                                "p (i c) -> p i c", c=2 * D))

                # ---- Phase C ----
                off_lo = 0   # in tiles, global across chunks
                off_hi = 0
                off_t = 0    # tile offset into qidx/occ
                qctr = 0     # swdge queue round-robin
                for ch in (chunks if 'noc' not in ablate else []):
                    c_lo = sum(layout[j][0] for j in ch)
                    c_hi = sum(layout[j][1] for j in ch)
                    tk = c_lo + c_hi

                    # metadata loads
                    oc_t = mp.tile([P, tk], f32, tag="oc")
                    nc.sync.dma_start(out=oc_t[:],
                                      in_=occ.ap()[:, off_t:off_t + tk])
                    qx_t = mp.tile([P, tk * 8], i16, tag="qx")
                    nc.sync.dma_start(out=qx_t[:],
                                      in_=qidx.ap()[:, off_t * 8:(off_t + tk) * 8])
                    if c_lo:
                        lx_t = mp.tile([P, c_lo * 8], i16, tag="lx")
                        nc.sync.dma_start(
                            out=lx_t[:],
                            in_=loidx.ap()[:, off_lo * 8:(off_lo + c_lo) * 8])
                    if c_hi:
                        hx_t = mp.tile([P, c_hi * 8], i16, tag="hx")
                        nc.sync.dma_start(
                            out=hx_t[:],
                            in_=hiidx.ap()[:, off_hi * 8:(off_hi + c_hi) * 8])

                    # gathers
                    kvg = gp.tile([P, tk * 2 * D], bf16, tag="kvg")
                    kvg3 = kvg[:].rearrange("p (t e) -> p t e", e=2 * D)
                    qg = gp.tile([P, tk * D], bf16, tag="qg")
                    qg3 = qg[:].rearrange("p (t e) -> p t e", e=D)
                    if 'nogather' in ablate:
                        for t in range(tk):
                            nc.sync.dma_start(out=kvg3[:, t, :],
                                              in_=kv_lo[0:P, :])
                            nc.sync.dma_start(out=qg3[:, t, :],
                                              in_=q_dram[0:P, :])
                    elif 'gone' in ablate:
                        # one full-out kv gather (wrong values, bounds-safe)
                        nc.gpsimd.dma_gather(
                            kvg3[:, :, :], kv_lo[:], qx_t[:],
                            tk * P, tk * P, 2 * D, single_packet=False)
                        nc.gpsimd.dma_gather(
                            qg3[:, :, :],
                            q_dram[:], qx_t[:], tk * P, tk * P, D, single_packet=False)
                    elif 'gsmall' in ablate:
                        nc.gpsimd.dma_gather(
                            kvg3[:, 0:1, :], kv_lo[:], lx_t[:, 0:8],
                            P, P, 2 * D, single_packet=False)
                        nc.gpsimd.dma_gather(
                            kvg3[:, 1:2, :], kv_hi[:], hx_t[:, 0:8],
                            P, P, 2 * D, single_packet=False)
                        nc.gpsimd.dma_gather(
                            qg3[:, 0:1, :],
                            q_dram[:], qx_t[:, 0:8], P, P, D,
                            single_packet=False)
                    elif 'gq' in ablate:
                        for t in range(tk):
                            nc.sync.dma_start(out=kvg3[:, t, :],
                                              in_=kv_lo[0:P, :])
                        nc.gpsimd.dma_gather(
                            qg3[:, :, :],
                            q_dram[:], qx_t[:], tk * P, tk * P, D, single_packet=False)
                    elif 'gkv' in ablate:
                        nc.gpsimd.dma_gather(
                            kvg3[:, :, :], kv_lo[:], qx_t[:],
                            tk * P, tk * P, 2 * D, single_packet=False)
                        for t in range(tk):
                            nc.sync.dma_start(out=qg3[:, t, :],
                                              in_=q_dram[0:P, :])
                    else:
                        # sub-gathers of <= SUBG tiles round-robin over the
                        # 4 SWDGE queues (parallel rings -> ~4x gather bw)
                        SUBG = 16
                        def subgather(dst3, base, ntile, tab, ix):
                            nonlocal qctr
                            for a in range(0, ntile, SUBG):
                                bN = min(SUBG, ntile - a)
                                nc.gpsimd.dma_gather(
                                    dst3[:, base + a:base + a + bN, :],
                                    tab[:], ix[:, a * 8:(a + bN) * 8],
                                    bN * P, bN * P, dst3.shape[-1],
                                    single_packet=False,
                                    queue_num=qctr % 4)
                                qctr += 1
                        if c_lo:
                            subgather(kvg3, 0, c_lo, kv_lo, lx_t)
                        if c_hi:
                            subgather(kvg3, c_lo, c_hi, kv_hi, hx_t)
                        subgather(qg3, 0, tk, q_dram, qx_t)

                    if 'gonly' in ablate:
                        off_lo += c_lo
                        off_hi += c_hi
                        off_t += tk
                        continue

                    # per-block psum accumulators
                    zt = {j: pz.tile([P, P], f32, tag=f"z{j % CHUNK}",
                                     name=f"zt{j % CHUNK}") for j in ch}
                    ntl = {j: pz.tile([P, P], f32, tag=f"n{j % CHUNK}",
                                      name=f"ntl{j % CHUNK}") for j in ch}

                    # tile -> (block, first?, last?) in chunk order
                    tmap = []
                    for reg in (0, 1):  # lo region then hi region
                        for j in ch:
                            tl, th = layout[j]
                            cnt = tl if reg == 0 else th
                            for _ in range(cnt):
                                tmap.append(j)
                    first_seen, last_idx = {}, {}
                    for t, j in enumerate(tmap):
                        first_seen.setdefault(j, t)
                        last_idx[j] = t

                    # process tiles in groups
                    t = 0
                    while t < tk:
                        g = min(GRP, tk - t)
                        t1 = wp.tile([P, GRP * D], bf16, tag="t1")
                        nc.vector.tensor_tensor(
                            out=t1[:, 0:g * D].rearrange("p (t c) -> p t c", t=g),
                            in0=kvg3[:, t:t + g, 0:D],
                            in1=qg3[:, t:t + g, :],
                            op=mybir.AluOpType.mult)
                        m_t = wp.tile([P, GRP * D], bf16, tag="m")
                        if 'noexp' in ablate:
                            nc.vector.tensor_copy(out=m_t[:, 0:g * D],
                                                  in_=t1[:, 0:g * D])
                        else:
                            nc.scalar.activation(
                                m_t[:, 0:g * D], t1[:, 0:g * D],
                                mybir.ActivationFunctionType.Exp,
                                bias=zb_s[:], scale=inv_sqrt_dk)
                        mv_t = wp.tile([P, GRP * D], bf16, tag="mv")
                        nc.vector.tensor_tensor(
                            out=mv_t[:, 0:g * D].rearrange("p (t c) -> p t c", t=g),
                            in0=m_t[:, 0:g * D].rearrange("p (t c) -> p t c", t=g),
                            in1=kvg3[:, t:t + g, D:2 * D],
                            op=mybir.AluOpType.mult)
                        s_t = wp.tile([P, GRP * P], bf16, tag="ssc")
                        if 'noss' in ablate:
                            pass
                        elif 'ssscalar' not in ablate:
                            nc.vector.tensor_tensor(
                                out=s_t[:, 0:g * P].rearrange(
                                    "p (t c) -> p t c", t=g),
                                in0=iF_s[:, 0:g * P].rearrange(
                                    "p (t c) -> p t c", t=g),
                                in1=oc_t[:, t:t + g].to_broadcast([P, g, P]),
                                op=mybir.AluOpType.is_equal)
                        else:
                            for i in range(g):
                                nc.vector.tensor_scalar(
                                    out=s_t[:, i * P:(i + 1) * P],
                                    in0=iF_s[:, 0:P],
                                    scalar1=oc_t[:, t + i:t + i + 1], scalar2=None,
                                    op0=mybir.AluOpType.is_equal)
                        if 'nomm' not in ablate:
                            src_s = iF_s if 'noss' in ablate else s_t
                            for i in range(g):
                                ti = t + i
                                j = tmap[ti]
                                st = first_seen[j] == ti
                                sp = last_idx[j] == ti
                                si = 0 if 'noss' in ablate else i * P
                                nc.tensor.matmul(out=zt[j][:],
                                                 lhsT=m_t[:, i * D:(i + 1) * D],
                                                 rhs=src_s[:, si:si + P],
                                                 start=st, stop=sp)
                                nc.tensor.matmul(out=ntl[j][:],
                                                 lhsT=mv_t[:, i * D:(i + 1) * D],
                                                 rhs=src_s[:, si:si + P],
                                                 start=st, stop=sp)
                        t += g

                    # epilogue per block
                    for j in (ch if ('noepi' not in ablate
                                     and 'nomm' not in ablate) else []):
                        rz = ep.tile([P, P], f32, tag="rz")
                        nc.vector.reciprocal(out=rz[:], in_=zt[j][:])
                        ox = ep.tile([P, P], bf16, tag="ox")
                        nc.vector.tensor_tensor(out=ox[:], in0=ntl[j][:],
                                                in1=rz[:],
                                                op=mybir.AluOpType.mult)
                        po = pp.tile([P, 2 * D], f32, tag="mm")
                        nc.tensor.matmul(out=po[:, 0:P], lhsT=wo_s[:], rhs=ox[:],
                                         start=True, stop=True)
                        o_sb = ep.tile([P, P], f32, tag="osb")
                        nc.vector.tensor_copy(out=o_sb[:], in_=po[:, 0:P])
                        nc.sync.dma_start(out=outT.ap()[:, j * P:(j + 1) * P],
                                          in_=o_sb[:])

                    off_lo += c_lo
                    off_hi += c_hi
                    off_t += tk

    nc.compile()
    _cache[key] = nc
    return nc


def _wrap_idx(idx_flat):
    """Pack a flat idx list into the [128, ceil(n/16)] int16 layout:
    idx i -> partition i%16, col i//16; replicated to the 8 groups of 16."""
    n = idx_flat.shape[0]
    cols = (n + 15) // 16
    arr = np.zeros((16, cols), np.int16)
    arr[
        np.arange(n) % 16, np.arange(n) // 16
    ] = idx_flat.astype(np.int16)
    return np.tile(arr, (8, 1))


def kernel(x, src, dst, Wq, bq, Wk, bk, Wv, bv, Wo, bo):
    x = np.asarray(x, dtype=np.float32)
    n, d = x.shape
    assert d == D
    src = np.asarray(src, dtype=np.int64)
    dst = np.asarray(dst, dtype=np.int64)

    n_all_blk = math.ceil(n / P)
    n_all_blk = math.ceil(n_all_blk / N_CORES) * N_CORES
    # ensure even number of blocks for lo/hi halves
    if n_all_blk % 2:
        n_all_blk += N_CORES
    n_pad = n_all_blk * P
    nblk_core = n_all_blk // N_CORES
    n_core = nblk_core * P
    half_blk = n_all_blk // 2
    half_rows = half_blk * P
    assert half_rows - 1 <= np.iinfo(np.int16).max
    assert n_core - 1 <= np.iinfo(np.int16).max

    # ---- host prep: sort edges by dst block, split by src half ----
    order = np.argsort(dst, kind="stable")
    sdst = dst[order]
    ssrc = src[order]
    blk = sdst // P
    counts = np.bincount(blk, minlength=n_all_blk)
    starts = np.zeros(n_all_blk + 1, dtype=np.int64)
    np.cumsum(counts, out=starts[1:])

    is_lo = ssrc < half_rows
    # per block lo/hi counts
    nlo = np.zeros(n_all_blk, np.int64)
    for b in range(n_all_blk):
        s0, s1 = starts[b], starts[b + 1]
        nlo[b] = int(is_lo[s0:s1].sum())
    nhi = counts - nlo
    tlo_b = (nlo + P - 1) // P
    thi_b = (nhi + P - 1) // P
    # uniform layout across cores: per block-slot j take max over cores
    tlo_j = [int(max(tlo_b[c * nblk_core + j] for c in range(N_CORES)))
             for j in range(nblk_core)]
    thi_j = [int(max(thi_b[c * nblk_core + j] for c in range(N_CORES)))
             for j in range(nblk_core)]
    layout = tuple((tlo_j[j], thi_j[j]) for j in range(nblk_core))
    total_tiles = sum(tl + th for tl, th in layout)
    total_lo = sum(tl for tl, th in layout)
    total_hi = sum(th for tl, th in layout)

    # chunk structure (must mirror _build)
    chunks = []
    j = 0
    while j < nblk_core:
        chunks.append(list(range(j, min(j + CHUNK, nblk_core))))
        j += CHUNK

    # ---- per-core data ----
    lo_np = np.zeros((N_CORES, P, total_lo * 8), np.int16)
    hi_np = np.zeros((N_CORES, P, max(total_hi, 1) * 8), np.int16)
    qx_np = np.zeros((N_CORES, P, total_tiles * 8), np.int16)
    oc_np = np.full((N_CORES, P, total_tiles), 255.0, np.float32)

    for c in range(N_CORES):
        lo_list, hi_list, q_list, oc_list = [], [], [], []
        # chunk-ordered
        for ch in chunks:
            blo, bhi, bq_, boc = [], [], [], []
            for reg in (0, 1):
                for j in ch:
                    b = c * nblk_core + j
                    s0, s1 = starts[b], starts[b + 1]
                    m = is_lo[s0:s1] if reg == 0 else ~is_lo[s0:s1]
                    es = ssrc[s0:s1][m]
                    ed = sdst[s0:s1][m]
                    ntile = layout[j][reg]
                    npad = ntile * P
                    idx = np.zeros(npad, np.int64)
                    idx[:es.shape[0]] = es if reg == 0 else es - half_rows
                    qi = np.zeros(npad, np.int64)
                    qi[:ed.shape[0]] = ed - c * n_core
                    ocv = np.full(npad, 255.0, np.float32)
                    ocv[:ed.shape[0]] = (ed - b * P).astype(np.float32)
                    (blo if reg == 0 else bhi).append(idx)
                    bq_.append(qi)
                    boc.append(ocv)
            lo_list.append(np.concatenate(blo) if blo else np.zeros(0, np.int64))
            hi_list.append(np.concatenate(bhi) if bhi else np.zeros(0, np.int64))
            q_list.append(np.concatenate(bq_))
            oc_list.append(np.concatenate(boc))
        lo_flat = np.concatenate(lo_list)
        hi_flat = np.concatenate(hi_list)
        q_flat = np.concatenate(q_list)
        oc_flat = np.concatenate(oc_list)
        # pack
        lo_np[c] = _wrap_idx_fill(lo_flat, total_lo * 8)
        hi_np[c] = _wrap_idx_fill(hi_flat, max(total_hi, 1) * 8)
        qx_np[c] = _wrap_idx_fill(q_flat, total_tiles * 8)
        # oc: [P, total_tiles]: slot (tile t, partition p) = edge t*128+p
        oc_np[c] = oc_flat.reshape(total_tiles, P).T

    xb = np.zeros((n_pad, D), np.float32)
    xb[:n] = x
    xT_np = np.ascontiguousarray(xb.T).astype(ml_dtypes.bfloat16)

    wkv_np = np.concatenate([np.asarray(Wk, np.float32),
                             np.asarray(Wv, np.float32)], axis=1) \
        .astype(ml_dtypes.bfloat16)
    wq_np = np.asarray(Wq, np.float32).astype(ml_dtypes.bfloat16)
    wo_np = np.asarray(Wo, np.float32).astype(ml_dtypes.bfloat16)
    bkv_np = np.concatenate([np.asarray(bk, np.float32),
                             np.asarray(bv, np.float32)])[None, :] \
        .astype(ml_dtypes.bfloat16)
    bq_np = np.asarray(bq, np.float32)[None, :].astype(ml_dtypes.bfloat16)
    ones1_np = np.ones((1, P), ml_dtypes.bfloat16)
    iota_np = np.tile(np.arange(P, dtype=np.float32)[None, :],
                      (P, GRP)).astype(ml_dtypes.bfloat16)
    with_bias = bool(np.any(np.asarray(bq)) or np.any(np.asarray(bk))
                     or np.any(np.asarray(bv)))

    import os
    nc = _build(layout, n_all_blk, half_rows, with_bias,
                ablate=os.environ.get("K_ABLATE", ""))

    in_maps = []
    for c in range(N_CORES):
        in_maps.append({
            "xT": xT_np,
            "xTq": np.ascontiguousarray(xT_np[:, c * n_core:(c + 1) * n_core]),
            "wkv": wkv_np, "wq": wq_np, "wo": wo_np,
            "bkv": bkv_np, "bqr": bq_np, "ones1": ones1_np, "iotaF": iota_np,
            "loidx": lo_np[c], "hiidx": hi_np[c], "qidx": qx_np[c],
            "occ": oc_np[c],
        })
    results = bass2jax.run_bass_via_pjrt(nc, in_maps, n_cores=N_CORES)

    out = np.empty((n_pad, D), np.float32)
    for c in range(N_CORES):
        out[c * n_core:(c + 1) * n_core] = results[c]["outT"].T
    out = out[:n] + np.asarray(bo, np.float32)[None, :]
    return out.astype(np.float32)


def _wrap_idx_fill(idx_flat, ncols):
    """_wrap_idx padded to exactly ncols columns."""
    arr = np.zeros((P, ncols), np.int16)
    if idx_flat.size == 0:
        return arr
    w = _wrap_idx(idx_flat)
    arr[:, :w.shape[1]] = w
    return arr


# revision 43
# speedup vs baseline: 3.1537x; 2.2168x over previous
"""Trainium2 Bass kernel: multi-head elementwise-attention GNN message passing.

Full inputs -> full output. Edges partitioned by destination-node block across
8 NeuronCores. Per core:
  Phase A: kv = [x@Wk | x@Wv] (+bias) in bf16 for ALL nodes, written to two
           DRAM tables (lo/hi node halves so gather indices fit int16).
  Phase B: q = x@Wq (+bias) in bf16 for the core's own nodes -> DRAM table.
  Phase C: per chunk of blocks, three dma_gather ops fetch per-edge k|v rows
           (by src) and q rows (by dst) into SBUF bf16; per tile of 128 edges:
           t1 = k*q (DVE 2x), m = exp(t1/4) (ACT), mv = m*v (DVE 2x),
           one-hot dst-offset columns via tensor_scalar is_equal (DVE 4x),
           z/num segment sums as bf16 matmuls accumulated in PSUM.
Out = (num/z) @ Wo done per block; bo added on host.
"""
import sys
sys.path.insert(0, '/opt/trn_rl_repo')
import math
import numpy as np
import ml_dtypes

import concourse.bass as bass
import concourse.bacc as bacc
import concourse.mybir as mybir
import concourse.tile as tile
from concourse import bass2jax

P = 128
D = 128
N_CORES = 8
CHUNK = 3  # dst blocks per gather chunk
GRP = 6    # tiles per DVE/ACT batch

f32 = mybir.dt.float32
bf16 = mybir.dt.bfloat16
i16 = mybir.dt.int16

_cache = {}


def _build(layout, n_all_blk, half_rows, with_bias, ablate='', repeat=1):
    """layout: tuple of (t_lo_j, t_hi_j) per owned block (uniform across cores).
    n_all_blk: total node blocks (kv table rows = n_all_blk*P, split lo/hi).
    half_rows: rows per kv half-table."""
    key = (layout, n_all_blk, half_rows, with_bias, ablate, repeat)
    if key in _cache:
        return _cache[key]
    nblk_core = len(layout)
    n_core = nblk_core * P
    total_tiles = sum(tl + th for tl, th in layout)
    total_lo = sum(tl for tl, th in layout)
    total_hi = sum(th for tl, th in layout)

    # chunk partitioning of the 49 blocks
    chunks = []
    j = 0
    while j < nblk_core:
        chunks.append(list(range(j, min(j + CHUNK, nblk_core))))
        j += CHUNK

    nc = bacc.Bacc("TRN2", target_bir_lowering=False, debug=False,
                   num_devices=N_CORES, num_swdge_queues=4)
    # ---- I/O ----
    xT = nc.dram_tensor("xT", [P, n_all_blk * P], bf16, kind="ExternalInput")
    wkv = nc.dram_tensor("wkv", [D, 2 * D], bf16, kind="ExternalInput")
    wq = nc.dram_tensor("wq", [D, D], bf16, kind="ExternalInput")
    wo = nc.dram_tensor("wo", [D, D], bf16, kind="ExternalInput")
    bkv = nc.dram_tensor("bkv", [1, 2 * D], bf16, kind="ExternalInput")
    bqr = nc.dram_tensor("bqr", [1, D], bf16, kind="ExternalInput")
    ones1 = nc.dram_tensor("ones1", [1, P], bf16, kind="ExternalInput")
    iotaF = nc.dram_tensor("iotaF", [P, GRP * P], bf16, kind="ExternalInput")
    loidx = nc.dram_tensor("loidx", [P, total_lo * 8], i16, kind="ExternalInput")
    hiidx = nc.dram_tensor("hiidx", [P, max(total_hi, 1) * 8], i16,
                           kind="ExternalInput")
    qidx = nc.dram_tensor("qidx", [P, total_tiles * 8], i16, kind="ExternalInput")
    occ = nc.dram_tensor("occ", [P, total_tiles], f32, kind="ExternalInput")
    outT = nc.dram_tensor("outT", [P, n_core], f32, kind="ExternalOutput")

    inv_sqrt_dk = 1.0 / math.sqrt(D // 8)  # d_k = 16

    with tile.TileContext(nc) as tc:
        with tc.tile_pool(name="const", bufs=1) as cp, \
             tc.tile_pool(name="dram", bufs=1, space="DRAM") as dp, \
             tc.tile_pool(name="xld", bufs=4) as xp, \
             tc.tile_pool(name="kvw", bufs=4) as kp, \
             tc.tile_pool(name="meta", bufs=2) as mp, \
             tc.tile_pool(name="gath", bufs=2) as gp, \
             tc.tile_pool(name="work", bufs=4) as wp, \
             tc.tile_pool(name="epi", bufs=3) as ep, \
             tc.tile_pool(name="mm", bufs=2, space="PSUM") as pp, \
             tc.tile_pool(name="acc", bufs=1, space="PSUM") as pz:

            # ---- constants ----
            wkv_s = cp.tile([D, 2 * D], bf16)
            nc.sync.dma_start(out=wkv_s[:], in_=wkv.ap())
            wq_s = cp.tile([D, D], bf16)
            nc.sync.dma_start(out=wq_s[:], in_=wq.ap())
            wo_s = cp.tile([D, D], bf16)
            nc.sync.dma_start(out=wo_s[:], in_=wo.ap())
            on_s = cp.tile([1, P], bf16)
            nc.sync.dma_start(out=on_s[:], in_=ones1.ap())
            bkv_s = cp.tile([1, 2 * D], bf16)
            nc.sync.dma_start(out=bkv_s[:], in_=bkv.ap())
            bq_s = cp.tile([1, D], bf16)
            nc.sync.dma_start(out=bq_s[:], in_=bqr.ap())
            iF_s = cp.tile([P, GRP * P], bf16)
            nc.sync.dma_start(out=iF_s[:], in_=iotaF.ap())
            zb_s = cp.tile([P, 1], f32)
            nc.vector.memset(zb_s[:], 0.0)

            kv_lo = dp.tile([half_rows, 2 * D], bf16)
            kv_hi = dp.tile([half_rows, 2 * D], bf16)
            q_dram = dp.tile([n_core, D], bf16)

            for _rep in range(repeat):
                # ---- Phase A: kv for ALL nodes -> DRAM lo/hi. xT arrives
                # PER-CORE PERMUTED with the core's own 49 blocks first, so q
                # (computed from the same x tiles for blocks < nblk_core)
                # lands in q_dram early and q-gathers overlap the rest. ----
                XCH = 8  # blocks per x load
                half_blk = half_rows // P
                for b0 in range(0, n_all_blk, XCH):
                    bn = min(XCH, n_all_blk - b0)
                    xt = xp.tile([P, XCH * P], bf16, tag="xt")
                    nc.sync.dma_start(
                        out=xt[:, 0:bn * P],
                        in_=xT.ap()[:, b0 * P:(b0 + bn) * P])
                    for b in range(b0, b0 + bn, 2):
                        pn = min(2, b0 + bn - b)
                        pkv = pp.tile([P, 4 * D], f32, tag="mm")
                        for k in range(pn):
                            nc.tensor.matmul(
                                out=pkv[:, k * 2 * D:(k + 1) * 2 * D],
                                lhsT=xt[:, (b - b0 + k) * P:(b - b0 + k + 1) * P],
                                rhs=wkv_s[:], start=True, stop=not with_bias)
                            if with_bias:
                                nc.tensor.matmul(
                                    out=pkv[:, k * 2 * D:(k + 1) * 2 * D],
                                    lhsT=on_s[:], rhs=bkv_s[:],
                                    start=False, stop=True)
                        kv_t = kp.tile([P, 4 * D], bf16, tag="kvw")
                        if (b // 2) % 2 == 0:
                            nc.vector.tensor_copy(out=kv_t[:, 0:pn * 2 * D],
                                                  in_=pkv[:, 0:pn * 2 * D])
                        else:
                            nc.scalar.copy(out=kv_t[:, 0:pn * 2 * D],
                                           in_=pkv[:, 0:pn * 2 * D])
                        # both blocks of a pair land in the same half-table
                        dst = kv_lo if b < half_blk else kv_hi
                        bb = b if b < half_blk else b - half_blk
                        nc.sync.dma_start(
                            out=dst[bb * P:(bb + pn) * P, :].rearrange(
                                "(i p) c -> p i c", p=P),
                            in_=kv_t[:, 0:pn * 2 * D].rearrange(
                                "p (i c) -> p i c", c=2 * D))
                        # q for the core's own blocks (permuted first)
                        qn = max(0, min(b + pn, nblk_core) - b)
                        if qn:
                            pq = pp.tile([P, 4 * D], f32, tag="mm")
                            for k in range(qn):
                                nc.tensor.matmul(
                                    out=pq[:, k * 2 * D:k * 2 * D + D],
                                    lhsT=xt[:, (b - b0 + k) * P:
                                            (b - b0 + k + 1) * P],
                                    rhs=wq_s[:], start=True,
                                    stop=not with_bias)
                                if with_bias:
                                    nc.tensor.matmul(
                                        out=pq[:, k * 2 * D:k * 2 * D + D],
                                        lhsT=on_s[:], rhs=bq_s[:],
                                        start=False, stop=True)
                            q_t = kp.tile([P, 2 * D], bf16, tag="qw")
                            for k in range(qn):
                                if (b // 2) % 2 == 0:
                                    nc.scalar.copy(
                                        out=q_t[:, k * D:(k + 1) * D],
                                        in_=pq[:, k * 2 * D:k * 2 * D + D])
                                else:
                                    nc.vector.tensor_copy(
                                        out=q_t[:, k * D:(k + 1) * D],
                                        in_=pq[:, k * 2 * D:k * 2 * D + D])
                            nc.sync.dma_start(
                                out=q_dram[b * P:(b + qn) * P, :].rearrange(
                                    "(i p) c -> p i c", p=P),
                                in_=q_t[:, 0:qn * D].rearrange(
                                    "p (i c) -> p i c", c=D))

                # ---- Phase C ----
                off_lo = 0   # in tiles, global across chunks
                off_hi = 0
                off_t = 0    # tile offset into qidx/occ
                qctr = 0     # swdge queue round-robin
                for ch in (chunks if 'noc' not in ablate else []):
                    c_lo = sum(layout[j][0] for j in ch)
                    c_hi = sum(layout[j][1] for j in ch)
                    tk = c_lo + c_hi

                    # metadata loads
                    oc_t = mp.tile([P, tk], f32, tag="oc")
                    nc.sync.dma_start(out=oc_t[:],
                                      in_=occ.ap()[:, off_t:off_t + tk])
                    qx_t = mp.tile([P, tk * 8], i16, tag="qx")
                    nc.sync.dma_start(out=qx_t[:],
                                      in_=qidx.ap()[:, off_t * 8:(off_t + tk) * 8])
                    if c_lo:
                        lx_t = mp.tile([P, c_lo * 8], i16, tag="lx")
                        nc.sync.dma_start(
                            out=lx_t[:],
                            in_=loidx.ap()[:, off_lo * 8:(off_lo + c_lo) * 8])
                    if c_hi:
                        hx_t = mp.tile([P, c_hi * 8], i16, tag="hx")
                        nc.sync.dma_start(
                            out=hx_t[:],
                            in_=hiidx.ap()[:, off_hi * 8:(off_hi + c_hi) * 8])

                    # gathers
                    kvg = gp.tile([P, tk * 2 * D], bf16, tag="kvg")
                    kvg3 = kvg[:].rearrange("p (t e) -> p t e", e=2 * D)
                    qg = gp.tile([P, tk * D], bf16, tag="qg")
                    qg3 = qg[:].rearrange("p (t e) -> p t e", e=D)
                    if 'nogather' in ablate:
                        for t in range(tk):
                            nc.sync.dma_start(out=kvg3[:, t, :],
                                              in_=kv_lo[0:P, :])
                            nc.sync.dma_start(out=qg3[:, t, :],
                                              in_=q_dram[0:P, :])
                    elif 'gone' in ablate:
                        # one full-out kv gather (wrong values, bounds-safe)
                        nc.gpsimd.dma_gather(
                            kvg3[:, :, :], kv_lo[:], qx_t[:],
                            tk * P, tk * P, 2 * D, single_packet=False)
                        nc.gpsimd.dma_gather(
                            qg3[:, :, :],
                            q_dram[:], qx_t[:], tk * P, tk * P, D, single_packet=False)
                    elif 'gsmall' in ablate:
                        nc.gpsimd.dma_gather(
                            kvg3[:, 0:1, :], kv_lo[:], lx_t[:, 0:8],
                            P, P, 2 * D, single_packet=False)
                        nc.gpsimd.dma_gather(
                            kvg3[:, 1:2, :], kv_hi[:], hx_t[:, 0:8],
                            P, P, 2 * D, single_packet=False)
                        nc.gpsimd.dma_gather(
                            qg3[:, 0:1, :],
                            q_dram[:], qx_t[:, 0:8], P, P, D,
                            single_packet=False)
                    elif 'gq' in ablate:
                        for t in range(tk):
                            nc.sync.dma_start(out=kvg3[:, t, :],
                                              in_=kv_lo[0:P, :])
                        nc.gpsimd.dma_gather(
                            qg3[:, :, :],
                            q_dram[:], qx_t[:], tk * P, tk * P, D, single_packet=False)
                    elif 'gkv' in ablate:
                        nc.gpsimd.dma_gather(
                            kvg3[:, :, :], kv_lo[:], qx_t[:],
                            tk * P, tk * P, 2 * D, single_packet=False)
                        for t in range(tk):
                            nc.sync.dma_start(out=qg3[:, t, :],
                                              in_=q_dram[0:P, :])
                    else:
                        # sub-gathers of <= SUBG tiles round-robin over the
                        # 4 SWDGE queues (parallel rings -> ~4x gather bw)
                        SUBG = 16
                        def subgather(dst3, base, ntile, tab, ix):
                            nonlocal qctr
                            for a in range(0, ntile, SUBG):
                                bN = min(SUBG, ntile - a)
                                nc.gpsimd.dma_gather(
                                    dst3[:, base + a:base + a + bN, :],
                                    tab[:], ix[:, a * 8:(a + bN) * 8],
                                    bN * P, bN * P, dst3.shape[-1],
                                    single_packet=False,
                                    queue_num=qctr % 4)
                                qctr += 1
                        if c_lo:
                            subgather(kvg3, 0, c_lo, kv_lo, lx_t)
                        if c_hi:
                            subgather(kvg3, c_lo, c_hi, kv_hi, hx_t)
                        subgather(qg3, 0, tk, q_dram, qx_t)

                    if 'gonly' in ablate:
                        off_lo += c_lo
                        off_hi += c_hi
                        off_t += tk
                        continue

                    # per-block psum accumulators
                    zt = {j: pz.tile([P, P], f32, tag=f"z{j % CHUNK}",
                                     name=f"zt{j % CHUNK}") for j in ch}
                    ntl = {j: pz.tile([P, P], f32, tag=f"n{j % CHUNK}",
                                      name=f"ntl{j % CHUNK}") for j in ch}

                    # tile -> (block, first?, last?) in chunk order
                    tmap = []
                    for reg in (0, 1):  # lo region then hi region
                        for j in ch:
                            tl, th = layout[j]
                            cnt = tl if reg == 0 else th
                            for _ in range(cnt):
                                tmap.append(j)
                    first_seen, last_idx = {}, {}
                    for t, j in enumerate(tmap):
                        first_seen.setdefault(j, t)
                        last_idx[j] = t

                    # process tiles in groups
                    t = 0
                    while t < tk:
                        g = min(GRP, tk - t)
                        t1 = wp.tile([P, GRP * D], bf16, tag="t1")
                        nc.vector.tensor_tensor(
                            out=t1[:, 0:g * D].rearrange("p (t c) -> p t c", t=g),
                            in0=kvg3[:, t:t + g, 0:D],
                            in1=qg3[:, t:t + g, :],
                            op=mybir.AluOpType.mult)
                        m_t = wp.tile([P, GRP * D], bf16, tag="m")
                        if 'noexp' in ablate:
                            nc.vector.tensor_copy(out=m_t[:, 0:g * D],
                                                  in_=t1[:, 0:g * D])
                        else:
                            nc.scalar.activation(
                                m_t[:, 0:g * D], t1[:, 0:g * D],
                                mybir.ActivationFunctionType.Exp,
                                bias=zb_s[:], scale=inv_sqrt_dk)
                        mv_t = wp.tile([P, GRP * D], bf16, tag="mv")
                        nc.vector.tensor_tensor(
                            out=mv_t[:, 0:g * D].rearrange("p (t c) -> p t c", t=g),
                            in0=m_t[:, 0:g * D].rearrange("p (t c) -> p t c", t=g),
                            in1=kvg3[:, t:t + g, D:2 * D],
                            op=mybir.AluOpType.mult)
                        s_t = wp.tile([P, GRP * P], bf16, tag="ssc")
                        if 'noss' in ablate:
                            pass
                        elif 'ssscalar' not in ablate:
                            nc.vector.tensor_tensor(
                                out=s_t[:, 0:g * P].rearrange(
                                    "p (t c) -> p t c", t=g),
                                in0=iF_s[:, 0:g * P].rearrange(
                                    "p (t c) -> p t c", t=g),
                                in1=oc_t[:, t:t + g].to_broadcast([P, g, P]),
                                op=mybir.AluOpType.is_equal)
                        else:
                            for i in range(g):
                                nc.vector.tensor_scalar(
                                    out=s_t[:, i * P:(i + 1) * P],
                                    in0=iF_s[:, 0:P],
                                    scalar1=oc_t[:, t + i:t + i + 1], scalar2=None,
                                    op0=mybir.AluOpType.is_equal)
                        if 'nomm' not in ablate:
                            src_s = iF_s if 'noss' in ablate else s_t
                            for i in range(g):
                                ti = t + i
                                j = tmap[ti]
                                st = first_seen[j] == ti
                                sp = last_idx[j] == ti
                                si = 0 if 'noss' in ablate else i * P
                                nc.tensor.matmul(out=zt[j][:],
                                                 lhsT=m_t[:, i * D:(i + 1) * D],
                                                 rhs=src_s[:, si:si + P],
                                                 start=st, stop=sp)
                                nc.tensor.matmul(out=ntl[j][:],
                                                 lhsT=mv_t[:, i * D:(i + 1) * D],
                                                 rhs=src_s[:, si:si + P],
                                                 start=st, stop=sp)
                        t += g

                    # epilogue per block
                    for j in (ch if ('noepi' not in ablate
                                     and 'nomm' not in ablate) else []):
                        rz = ep.tile([P, P], f32, tag="rz")
                        nc.vector.reciprocal(out=rz[:], in_=zt[j][:])
                        ox = ep.tile([P, P], bf16, tag="ox")
                        nc.vector.tensor_tensor(out=ox[:], in0=ntl[j][:],
                                                in1=rz[:],
                                                op=mybir.AluOpType.mult)
                        po = pp.tile([P, 2 * D], f32, tag="mm")
                        nc.tensor.matmul(out=po[:, 0:P], lhsT=wo_s[:], rhs=ox[:],
                                         start=True, stop=True)
                        o_sb = ep.tile([P, P], f32, tag="osb")
                        nc.vector.tensor_copy(out=o_sb[:], in_=po[:, 0:P])
                        nc.sync.dma_start(out=outT.ap()[:, j * P:(j + 1) * P],
                                          in_=o_sb[:])

                    off_lo += c_lo
                    off_hi += c_hi
                    off_t += tk

    nc.compile()
    _cache[key] = nc
    return nc


def _wrap_idx(idx_flat):
    """Pack a flat idx list into the [128, ceil(n/16)] int16 layout:
    idx i -> partition i%16, col i//16; replicated to the 8 groups of 16."""
    n = idx_flat.shape[0]
    cols = (n + 15) // 16
    arr = np.zeros((16, cols), np.int16)
    arr[
        np.arange(n) % 16, np.arange(n) // 16
    ] = idx_flat.astype(np.int16)
    return np.tile(arr, (8, 1))


def kernel(x, src, dst, Wq, bq, Wk, bk, Wv, bv, Wo, bo):
    x = np.asarray(x, dtype=np.float32)
    n, d = x.shape
    assert d == D
    src = np.asarray(src, dtype=np.int64)
    dst = np.asarray(dst, dtype=np.int64)

    n_all_blk = math.ceil(n / P)
    n_all_blk = math.ceil(n_all_blk / N_CORES) * N_CORES
    # ensure even number of blocks for lo/hi halves
    if n_all_blk % 2:
        n_all_blk += N_CORES
    n_pad = n_all_blk * P
    nblk_core = n_all_blk // N_CORES
    n_core = nblk_core * P
    half_blk = n_all_blk // 2
    half_rows = half_blk * P
    assert half_rows - 1 <= np.iinfo(np.int16).max
    assert n_core - 1 <= np.iinfo(np.int16).max

    # ---- host prep: sort edges by dst block, split by src half ----
    order = np.argsort(dst, kind="stable")
    sdst = dst[order]
    ssrc = src[order]
    blk = sdst // P
    counts = np.bincount(blk, minlength=n_all_blk)
    starts = np.zeros(n_all_blk + 1, dtype=np.int64)
    np.cumsum(counts, out=starts[1:])

    # per-core block permutation: own blocks first, then the rest in order.
    # permpos[c][orig_block] = position in core c's permuted table; the kv
    # table row of node s on core c is permpos[c][s//P]*P + s%P.
    perm = np.empty((N_CORES, n_all_blk), np.int64)
    permpos = np.empty((N_CORES, n_all_blk), np.int64)
    for c in range(N_CORES):
        own = np.arange(c * nblk_core, (c + 1) * nblk_core)
        rest = np.concatenate([np.arange(0, c * nblk_core),
                               np.arange((c + 1) * nblk_core, n_all_blk)])
        perm[c] = np.concatenate([own, rest])
        permpos[c][perm[c]] = np.arange(n_all_blk)

    # per-core lo/hi split of each owned block's edges
    srow = np.empty((N_CORES, ssrc.shape[0]), np.int64)
    for c in range(N_CORES):
        srow[c] = permpos[c][ssrc // P] * P + (ssrc % P)
    is_lo_c = srow < half_rows
    tlo_b = np.zeros((N_CORES, nblk_core), np.int64)
    thi_b = np.zeros((N_CORES, nblk_core), np.int64)
    for c in range(N_CORES):
        for j in range(nblk_core):
            b = c * nblk_core + j
            s0, s1 = starts[b], starts[b + 1]
            nl = int(is_lo_c[c, s0:s1].sum())
            nh = int(s1 - s0) - nl
            tlo_b[c, j] = (nl + P - 1) // P
            thi_b[c, j] = (nh + P - 1) // P
    layout = tuple((int(tlo_b[:, j].max()), int(thi_b[:, j].max()))
                   for j in range(nblk_core))
    total_tiles = sum(tl + th for tl, th in layout)
    total_lo = sum(tl for tl, th in layout)
    total_hi = sum(th for tl, th in layout)

    # chunk structure (must mirror _build)
    chunks = []
    j = 0
    while j < nblk_core:
        chunks.append(list(range(j, min(j + CHUNK, nblk_core))))
        j += CHUNK

    # ---- per-core data ----
    lo_np = np.zeros((N_CORES, P, total_lo * 8), np.int16)
    hi_np = np.zeros((N_CORES, P, max(total_hi, 1) * 8), np.int16)
    qx_np = np.zeros((N_CORES, P, total_tiles * 8), np.int16)
    oc_np = np.full((N_CORES, P, total_tiles), 255.0, np.float32)

    for c in range(N_CORES):
        lo_list, hi_list, q_list, oc_list = [], [], [], []
        # chunk-ordered
        for ch in chunks:
            blo, bhi, bq_, boc = [], [], [], []
            for reg in (0, 1):
                for j in ch:
                    b = c * nblk_core + j
                    s0, s1 = starts[b], starts[b + 1]
                    m = is_lo_c[c, s0:s1] if reg == 0 else ~is_lo_c[c, s0:s1]
                    es = srow[c, s0:s1][m]
                    ed = sdst[s0:s1][m]
                    ntile = layout[j][reg]
                    npad = ntile * P
                    idx = np.zeros(npad, np.int64)
                    idx[:es.shape[0]] = es if reg == 0 else es - half_rows
                    qi = np.zeros(npad, np.int64)
                    qi[:ed.shape[0]] = ed - c * n_core
                    ocv = np.full(npad, 255.0, np.float32)
                    ocv[:ed.shape[0]] = (ed - b * P).astype(np.float32)
                    (blo if reg == 0 else bhi).append(idx)
                    bq_.append(qi)
                    boc.append(ocv)
            lo_list.append(np.concatenate(blo) if blo else np.zeros(0, np.int64))
            hi_list.append(np.concatenate(bhi) if bhi else np.zeros(0, np.int64))
            q_list.append(np.concatenate(bq_))
            oc_list.append(np.concatenate(boc))
        lo_flat = np.concatenate(lo_list)
        hi_flat = np.concatenate(hi_list)
        q_flat = np.concatenate(q_list)
        oc_flat = np.concatenate(oc_list)
        # pack
        lo_np[c] = _wrap_idx_fill(lo_flat, total_lo * 8)
        hi_np[c] = _wrap_idx_fill(hi_flat, max(total_hi, 1) * 8)
        qx_np[c] = _wrap_idx_fill(q_flat, total_tiles * 8)
        # oc: [P, total_tiles]: slot (tile t, partition p) = edge t*128+p
        oc_np[c] = oc_flat.reshape(total_tiles, P).T

    xb = np.zeros((n_pad, D), np.float32)
    xb[:n] = x
    xT_np = np.ascontiguousarray(xb.T).astype(ml_dtypes.bfloat16)

    wkv_np = np.concatenate([np.asarray(Wk, np.float32),
                             np.asarray(Wv, np.float32)], axis=1) \
        .astype(ml_dtypes.bfloat16)
    wq_np = np.asarray(Wq, np.float32).astype(ml_dtypes.bfloat16)
    wo_np = np.asarray(Wo, np.float32).astype(ml_dtypes.bfloat16)
    bkv_np = np.concatenate([np.asarray(bk, np.float32),
                             np.asarray(bv, np.float32)])[None, :] \
        .astype(ml_dtypes.bfloat16)
    bq_np = np.asarray(bq, np.float32)[None, :].astype(ml_dtypes.bfloat16)
    ones1_np = np.ones((1, P), ml_dtypes.bfloat16)
    iota_np = np.tile(np.arange(P, dtype=np.float32)[None, :],
                      (P, GRP)).astype(ml_dtypes.bfloat16)
    with_bias = bool(np.any(np.asarray(bq)) or np.any(np.asarray(bk))
                     or np.any(np.asarray(bv)))

    import os
    nc = _build(layout, n_all_blk, half_rows, with_bias,
                ablate=os.environ.get("K_ABLATE", ""))

    # per-core permuted xT (own blocks first)
    xT_blocks = xT_np.reshape(P, n_all_blk, P)
    in_maps = []
    for c in range(N_CORES):
        xT_c = np.ascontiguousarray(
            xT_blocks[:, perm[c], :]).reshape(P, n_pad)
        in_maps.append({
            "xT": xT_c,
            "wkv": wkv_np, "wq": wq_np, "wo": wo_np,
            "bkv": bkv_np, "bqr": bq_np, "ones1": ones1_np, "iotaF": iota_np,
            "loidx": lo_np[c], "hiidx": hi_np[c], "qidx": qx_np[c],
            "occ": oc_np[c],
        })
    results = bass2jax.run_bass_via_pjrt(nc, in_maps, n_cores=N_CORES)

    out = np.empty((n_pad, D), np.float32)
    for c in range(N_CORES):
        out[c * n_core:(c + 1) * n_core] = results[c]["outT"].T
    out = out[:n] + np.asarray(bo, np.float32)[None, :]
    return out.astype(np.float32)


def _wrap_idx_fill(idx_flat, ncols):
    """_wrap_idx padded to exactly ncols columns."""
    arr = np.zeros((P, ncols), np.int16)
    if idx_flat.size == 0:
        return arr
    w = _wrap_idx(idx_flat)
    arr[:, :w.shape[1]] = w
    return arr
